# revision 23
# baseline (speedup 1.0000x reference)
"""GalerkinBlock Trainium2 Bass kernel.

B=4, N=8192, C=512, H=4, D=128, HID=2048. 8 NeuronCores.

Sharding: cores (2b, 2b+1) handle batch b; core c owns rows
[(c%2)*4096, (c%2+1)*4096) of its batch. The only cross-core coupling is
context = k^T v (a sum over N), reduced with a pairwise AllReduce of a
[128, 520] f32 buffer (4 heads' [D,D] partial contexts + per-head column
sums of k_std / v_std needed for the general lnk/lnv affine correction).

All layernorm affine weights are folded on the host:
  LN1 (w,b)  -> qkv_w / qkv bias
  LN2 (w,b)  -> mlp_w1 / mlp bias
  lnk/lnv    -> post-AllReduce correction of the [D,D] context:
     ctx = S*wk wv^T (.) M  +  S*(wk.sk + N bk) bv^T  +  S*bk (wv.sv)^T
     (M = ks^T vs, sk = ks^T 1, sv = vs^T 1, S = D**-0.5; all terms exact)
so on-device layernorms are pure standardization (bn_stats + affine apply).

Matmul layout: activations are transposed once per LN (PE transpose of the
bf16 standardized tile); q^T, attn^T, mid^T are produced directly in
transposed form by swapping the stationary/moving operands, so no other
transposes are needed.
"""

import numpy as np

B, N, C = 4, 8192, 512
H = 4
D = C // H          # 128
HID = 4 * C         # 2048
SCALE = D ** -0.5
EPS = 1e-5

NCORES = 8
RPC = (B * N) // NCORES      # rows per core = 4096
P = 128                      # partition / row-tile size
NT = RPC // P                # 32 row tiles per core
NG = RPC // 512              # 8 groups of 512 rows
CC = C // P                  # 4 c-chunks
HT = HID // P                # 16 hid chunks

_cache = {}


def _build_nc(num_cores, debug=False):
    import concourse.bass as bass
    import concourse.tile as tile
    from concourse import bacc, mybir

    f32 = mybir.dt.float32
    bf16 = mybir.dt.bfloat16
    AF = mybir.ActivationFunctionType
    OP = mybir.AluOpType

    # Bacc (not plain Bass): its finalize() pipeline splits semaphore waits
    # into EventSemaphore instructions (TRN2 allows <=1 wait per instruction)
    nc = bacc.Bacc(num_devices=num_cores)

    # ---- I/O ----
    xs = nc.dram_tensor("xs", [RPC, C], f32, kind="ExternalInput")
    wkv = nc.dram_tensor("wkv", [C, 2 * C], bf16, kind="ExternalInput")
    wq = nc.dram_tensor("wq", [C, C], bf16, kind="ExternalInput")
    wp = nc.dram_tensor("wp", [C, C], bf16, kind="ExternalInput")
    w1 = nc.dram_tensor("w1", [C, HID], bf16, kind="ExternalInput")
    w2 = nc.dram_tensor("w2", [HID, C], bf16, kind="ExternalInput")
    ckv_r = nc.dram_tensor("ckv_r", [1, 2 * C], bf16, kind="ExternalInput")
    cq_c = nc.dram_tensor("cq_c", [P, H], f32, kind="ExternalInput")
    c1_c = nc.dram_tensor("c1_c", [P, HT], f32, kind="ExternalInput")
    pb_r = nc.dram_tensor("pb_r", [1, C], bf16, kind="ExternalInput")
    b2_r = nc.dram_tensor("b2_r", [1, C], bf16, kind="ExternalInput")
    ones_r = nc.dram_tensor("ones_r", [1, P], bf16, kind="ExternalInput")
    ones_c = nc.dram_tensor("ones_c", [P, 1], bf16, kind="ExternalInput")
    ident = nc.dram_tensor("ident", [P, P], bf16, kind="ExternalInput")
    swk_c = nc.dram_tensor("swk_c", [P, 1], f32, kind="ExternalInput")
    snbk_c = nc.dram_tensor("snbk_c", [P, 1], f32, kind="ExternalInput")
    wv_c = nc.dram_tensor("wv_c", [P, 1], f32, kind="ExternalInput")
    sbk_r = nc.dram_tensor("sbk_r", [1, P], bf16, kind="ExternalInput")
    bv_r = nc.dram_tensor("bv_r", [1, P], bf16, kind="ExternalInput")
    wv4_r = nc.dram_tensor("wv4_r", [1, C], bf16, kind="ExternalInput")
    out = nc.dram_tensor("out", [RPC, C], f32, kind="ExternalOutput")
    if debug:
        dbg = {n: nc.dram_tensor(n, s, f32, kind="ExternalOutput")
               for n, s in [("d_z", [P, C]), ("d_kvh", [P, 2 * C]),
                            ("d_ctxs", [P, C]), ("d_q", [P, 512]),
                            ("d_at", [P, 512]), ("d_x1", [P, C]),
                            ("d_mid", [P, 512]), ("d_M", [P, C + 2 * H])]}

    groups = [[2 * i, 2 * i + 1] for i in range(num_cores // 2)]

    with tile.TileContext(nc) as tc:
        from contextlib import ExitStack

        def _dma(out_ap, in_ap):
            return nc.gpsimd.dma_start(out_ap, in_ap)

        def _collect(pool):
            pass

        with ExitStack() as ctx:
            wpool = ctx.enter_context(tc.tile_pool(name="weights", bufs=1))
            zpool = ctx.enter_context(tc.tile_pool(name="zstore", bufs=1))
            spool = ctx.enter_context(tc.tile_pool(name="stats", bufs=4))
            cpool = ctx.enter_context(tc.tile_pool(name="collect", bufs=16))
            dram = ctx.enter_context(tc.tile_pool(name="dram", bufs=1, space="DRAM"))

            # ---- persistent weights in SBUF ----
            wkv_sb = wpool.tile([P, CC, 2 * C], bf16)   # rhs chunks for k,v
            wq_sb = wpool.tile([P, CC, C], bf16)        # lhsT chunks for q^T
            wp_sb = wpool.tile([P, CC, C], bf16)        # rhs chunks for proj
            w1_sb = wpool.tile([P, CC, HID], bf16)      # lhsT chunks for mid^T
            w2_sb = wpool.tile([P, HT, C], bf16)        # rhs chunks for mlp2
            for cc in range(CC):
                _dma(wkv_sb[:, cc, :], wkv[cc * P:(cc + 1) * P, :])
                _dma(wq_sb[:, cc, :], wq[cc * P:(cc + 1) * P, :])
                _dma(wp_sb[:, cc, :], wp[cc * P:(cc + 1) * P, :])
                _dma(w1_sb[:, cc, :], w1[cc * P:(cc + 1) * P, :])
            for t in range(HT):
                _dma(w2_sb[:, t, :], w2[t * P:(t + 1) * P, :])

            ckv_sb = wpool.tile([1, 2 * C], bf16)
            cq_sb = wpool.tile([P, H], f32)
            c1_sb = wpool.tile([P, HT], f32)
            pb_sb = wpool.tile([1, C], bf16)
            b2_sb = wpool.tile([1, C], bf16)
            ones_r_sb = wpool.tile([1, P], bf16)
            ones_c_sb = wpool.tile([P, 1], bf16)
            ident_sb = wpool.tile([P, P], bf16)
            eps_sb = wpool.tile([P, 1], f32)
            nc.vector.memset(eps_sb[:], EPS)
            swk_sb = wpool.tile([P, 1], f32)
            snbk_sb = wpool.tile([P, 1], f32)
            wvc_sb = wpool.tile([P, 1], f32)
            sbk_sb = wpool.tile([1, P], bf16)
            bv_sb = wpool.tile([1, P], bf16)
            wv4_sb = wpool.tile([1, C], bf16)
            for sb, dr in [(ckv_sb, ckv_r), (cq_sb, cq_c), (c1_sb, c1_c),
                           (pb_sb, pb_r), (b2_sb, b2_r), (ones_r_sb, ones_r),
                           (ones_c_sb, ones_c), (ident_sb, ident),
                           (swk_sb, swk_c), (snbk_sb, snbk_c), (wvc_sb, wv_c),
                           (sbk_sb, sbk_r), (bv_sb, bv_r), (wv4_sb, wv4_r)]:
                _dma(sb[:], dr[:])

            # z^T storage: [c-part within chunk, c-chunk, row] bf16
            zT = zpool.tile([P, CC, RPC], bf16)
            _collect(cpool)

            # =========================== PHASE A ===========================
            with ExitStack() as actx:
                acc = actx.enter_context(
                    tc.tile_pool(name="acc_ps", bufs=1, space="PSUM"))
                ctx_ps = acc.tile([P, C], f32)        # 4 heads' [D,D] partials
                sksv_ps = acc.tile([P, 2 * H], f32)   # sk | sv column sums

                apool = actx.enter_context(tc.tile_pool(name="awork", bufs=3))
                kvpool = actx.enter_context(tc.tile_pool(name="kvhat", bufs=2))
                pkv = actx.enter_context(
                    tc.tile_pool(name="kv_ps", bufs=2, space="PSUM"))
                pzt = actx.enter_context(
                    tc.tile_pool(name="zt_ps", bufs=2, space="PSUM"))

                for i in range(NT):
                    r0 = i * P
                    x_t = apool.tile([P, C], f32, tag="x")
                    _dma(x_t[:], xs[r0:r0 + P, :])

                    # LN1 standardization (stats fp32)
                    st6 = spool.tile([P, 6], f32, tag="st6")
                    nc.vector.bn_stats(st6[:], x_t[:])
                    mv = spool.tile([P, 2], f32, tag="mv")
                    nc.vector.bn_aggr(mv[:], st6[:])
                    rstd = spool.tile([P, 1], f32, tag="rstd")
                    nc.scalar.activation(rstd[:], mv[:, 1:2], AF.Sqrt, bias=eps_sb[:])
                    nc.vector.reciprocal(rstd[:], rstd[:])
                    z_t = apool.tile([P, C], bf16, tag="z")
                    nc.vector.tensor_scalar(
                        z_t[:], x_t[:], scalar1=mv[:, 0:1], scalar2=rstd[:],
                        op0=OP.subtract, op1=OP.mult)

                    # transpose z -> zT (PE), evacuate to persistent store
                    zt_ps = pzt.tile([P, C], bf16)
                    for cc in range(CC):
                        nc.tensor.transpose(
                            zt_ps[:, cc * P:(cc + 1) * P],
                            z_t[:, cc * P:(cc + 1) * P], ident_sb[:])
                    nc.scalar.copy(
                        zT[:, :, r0:r0 + P],
                        zt_ps[:].rearrange("p (c r) -> p c r", c=CC))

                    # k|v = z @ Wkv + ckv   (psum [rows, 2C])
                    kv_ps = pkv.tile([P, 2 * C], f32)
                    for half in range(2):
                        sl = slice(half * C, half * C + C)
                        for cc in range(CC):
                            nc.tensor.matmul(
                                kv_ps[:, sl], zT[:, cc, r0:r0 + P],
                                wkv_sb[:, cc, sl],
                                start=(cc == 0), stop=False)
                        nc.tensor.matmul(
                            kv_ps[:, sl], ones_r_sb[:], ckv_sb[:, sl],
                            start=False, stop=True)

                    # per-head standardization of k and v (8 heads' worth)
                    st48 = spool.tile([P, 8, 6], f32, tag="st48")
                    mv8 = spool.tile([P, 8, 2], f32, tag="mv8")
                    for s in range(2 * H):
                        nc.vector.bn_stats(st48[:, s, :],
                                           kv_ps[:, s * D:(s + 1) * D])
                        nc.vector.bn_aggr(mv8[:, s, :], st48[:, s, :])
                    rstd8 = spool.tile([P, 8], f32, tag="rstd8")
                    nc.scalar.activation(rstd8[:], mv8[:, :, 1], AF.Sqrt, bias=eps_sb[:])
                    nc.vector.reciprocal(rstd8[:], rstd8[:])
                    nmr8 = spool.tile([P, 8], f32, tag="nmr8")
                    nc.vector.tensor_tensor(nmr8[:], mv8[:, :, 0], rstd8[:],
                                            op=OP.mult)
                    nc.vector.tensor_scalar(nmr8[:], nmr8[:], scalar1=-1.0,
                                            scalar2=None, op0=OP.mult)
                    if debug and i == 0:
                        zf = apool.tile([P, C], f32, tag="dzf")
                        nc.vector.tensor_copy(zf[:], z_t[:])
                        _dma(dbg["d_z"][:], zf[:])
                    kvh = kvpool.tile([P, 2 * C], bf16, tag="kvh")
                    for s in range(2 * H):
                        nc.scalar.activation(
                            kvh[:, s * D:(s + 1) * D], kv_ps[:, s * D:(s + 1) * D],
                            AF.Identity, bias=nmr8[:, s:s + 1],
                            scale=rstd8[:, s:s + 1])

                    if debug and i == 0:
                        kvf = apool.tile([P, 2 * C], f32, tag="dkvf")
                        nc.vector.tensor_copy(kvf[:], kvh[:])
                        _dma(dbg["d_kvh"][:], kvf[:])
                    # context partial + column sums (accumulate over tiles)
                    # NOTE: start=True clears has_written for the WHOLE
                    # psum bank, so only the very first matmul touching each
                    # accumulator bank may use start=True.
                    for h in range(H):
                        ks = kvh[:, h * D:(h + 1) * D]
                        vs = kvh[:, C + h * D:C + (h + 1) * D]
                        nc.tensor.matmul(
                            ctx_ps[:, h * D:(h + 1) * D], ks, vs,
                            start=(i == 0 and h == 0),
                            stop=(i == NT - 1 and h == H - 1),
                            skip_group_check=True)
                        nc.tensor.matmul(
                            sksv_ps[:, h:h + 1], ks, ones_c_sb[:],
                            start=(i == 0 and h == 0),
                            stop=(i == NT - 1 and h == H - 1),
                            skip_group_check=True)
                        nc.tensor.matmul(
                            sksv_ps[:, H + h:H + h + 1], vs, ones_c_sb[:],
                            start=(i == 0 and h == 0),
                            stop=(i == NT - 1 and h == H - 1),
                            skip_group_check=True)

                # ---- evacuate accumulators, AllReduce ----
                ar_sb = apool.tile([P, C + 2 * H], f32, tag="ar")
                nc.scalar.copy(ar_sb[:, 0:C], ctx_ps[:])
                nc.vector.tensor_copy(ar_sb[:, C:C + 2 * H], sksv_ps[:])
                ar_in = dram.tile([P, C + 2 * H], f32)
                ar_out = dram.tile([P, C + 2 * H], f32)
                _dma(ar_in[:], ar_sb[:])
                nc.gpsimd.collective_compute(
                    "AllReduce", OP.add, replica_groups=groups,
                    ins=[ar_in[:]], outs=[ar_out[:]])

            # ====================== context assembly =======================
            # (everything below the AllReduce DMA-in depends on the collective)
            gctx = ctx.enter_context(__import__("contextlib").ExitStack())
            gpool = ctx.enter_context(tc.tile_pool(name="gwork", bufs=1))
            pg = gctx.enter_context(tc.tile_pool(name="g_ps", bufs=1, space="PSUM"))

            # WVB = ones (x) wv4  (broadcast of tiled lnv_w), no AR dependency
            wvb_ps = pg.tile([P, C], f32)
            nc.tensor.matmul(wvb_ps[:], ones_r_sb[:], wv4_sb[:])

            M_sb = gpool.tile([P, C + 2 * H], f32)
            _dma(M_sb[:], ar_out[:])

            # term1 = (S*wk)[d] * M[d,e] * wv[e]
            t1 = gpool.tile([P, C], f32)
            nc.vector.tensor_tensor(t1[:], M_sb[:, 0:C], wvb_ps[:], op=OP.mult)
            nc.vector.tensor_scalar(t1[:], t1[:], scalar1=swk_sb[:],
                                    scalar2=None, op0=OP.mult)

            # gamma = (S*wk)*sk + S*N*bk ; eps_v = wv*sv   (both [P, H])
            ge8 = gpool.tile([P, 2 * H], f32)
            nc.vector.tensor_scalar(
                ge8[:, 0:H], M_sb[:, C:C + H], scalar1=swk_sb[:],
                scalar2=snbk_sb[:], op0=OP.mult, op1=OP.add)
            nc.vector.tensor_scalar(
                ge8[:, H:2 * H], M_sb[:, C + H:C + 2 * H], scalar1=wvc_sb[:],
                scalar2=None, op0=OP.mult)
            ge8b = gpool.tile([P, 2 * H], bf16)
            nc.vector.tensor_copy(ge8b[:], ge8[:])
            # engines can't address partition offsets 1..7, so transpose each
            # [128,1] column separately into partition 0 of its own slot
            geT_ps = pg.tile([1, 2 * H * P], bf16)
            for s in range(2 * H):
                nc.tensor.transpose(geT_ps[0:1, s * P:(s + 1) * P],
                                    ge8b[:, s:s + 1], ident_sb[:])
            geT = gpool.tile([1, 2 * H * P], bf16)
            nc.scalar.copy(geT[:], geT_ps[:])

            # corr = gamma (x) bv + (S*bk) (x) eps_v     per head
            corr_ps = pg.tile([P, C], f32)
            for h in range(H):
                sl = slice(h * D, (h + 1) * D)
                nc.tensor.matmul(corr_ps[:, sl], geT[0:1, h * P:(h + 1) * P],
                                 bv_sb[:], start=(h == 0), stop=False)
                nc.tensor.matmul(corr_ps[:, sl], sbk_sb[:],
                                 geT[0:1, (H + h) * P:(H + h + 1) * P],
                                 start=False, stop=(h == H - 1))
            ctxs = gpool.tile([P, C], bf16)
            nc.vector.tensor_tensor(ctxs[:], t1[:], corr_ps[:], op=OP.add)

            if debug:
                _dma(dbg["d_M"][:], M_sb[:])
                cxf = gpool.tile([P, C], f32)
                nc.vector.tensor_copy(cxf[:], ctxs[:])
                _dma(dbg["d_ctxs"][:], cxf[:])
            gctx.close()
            _collect(cpool)

            # =========================== PHASE B ===========================
            with ExitStack() as bctx:
                bpool = bctx.enter_context(tc.tile_pool(name="bwork", bufs=3))
                qpool = bctx.enter_context(tc.tile_pool(name="qt_sb", bufs=12))
                atpool = bctx.enter_context(tc.tile_pool(name="at_sb", bufs=8))
                x1pool = bctx.enter_context(tc.tile_pool(name="x1_sb", bufs=6))
                htpool = bctx.enter_context(tc.tile_pool(name="ht_sb", bufs=2))
                midpool = bctx.enter_context(tc.tile_pool(name="mid_sb", bufs=1))
                pf2 = bctx.enter_context(
                    tc.tile_pool(name="f2_ps", bufs=2, space="PSUM"))
                pf1 = bctx.enter_context(
                    tc.tile_pool(name="f1_ps", bufs=1, space="PSUM"))
                pht = bctx.enter_context(
                    tc.tile_pool(name="ht_ps", bufs=1, space="PSUM"))
                pmid = bctx.enter_context(
                    tc.tile_pool(name="mid_ps", bufs=2, space="PSUM"))

                def emit_qT(g):
                    """q^T for group g: psum [dq, 512 rows] per head."""
                    g0 = g * 512
                    tiles = []
                    for h in range(H):
                        q_ps = pf2.tile([P, 512], f32, tag="qt_ps")
                        for cc in range(CC):
                            nc.tensor.matmul(
                                q_ps[:], wq_sb[:, cc, h * D:(h + 1) * D],
                                zT[:, cc, g0:g0 + 512],
                                start=(cc == 0), stop=(cc == CC - 1))
                        q_sb = qpool.tile([P, 512], bf16, tag="qt")
                        nc.scalar.activation(q_sb[:], q_ps[:], AF.Identity,
                                             bias=cq_sb[:, h:h + 1])
                        tiles.append(q_sb)
                    return tiles

                qT_pre = {g: emit_qT(g) for g in range(2)}  # overlap AllReduce

                for g in range(NG):
                    g0 = g * 512
                    qT = qT_pre[g] if g in qT_pre else emit_qT(g)

                    # attn^T per head: lhsT = ctxs_h, rhs = qT_h
                    atT = []
                    for h in range(H):
                        a_ps = pf1.tile([P, 512], f32, tag="at_ps")
                        nc.tensor.matmul(a_ps[:], ctxs[:, h * D:(h + 1) * D],
                                         qT[h][:])
                        a_sb = atpool.tile([P, 512], bf16, tag="at")
                        nc.scalar.copy(a_sb[:], a_ps[:])
                        atT.append(a_sb)
                    if debug and g == 0:
                        qf = bpool.tile([P, 512], f32, tag="dqf")
                        nc.vector.tensor_copy(qf[:], qT[0][:])
                        _dma(dbg["d_q"][:], qf[:])
                        af = bpool.tile([P, 512], f32, tag="daf")
                        nc.vector.tensor_copy(af[:], atT[0][:])
                        _dma(dbg["d_at"][:], af[:])

                    x1_t = []
                    hT_g = htpool.tile([P, CC, 512], bf16, tag="htg")
                    for j in range(4):
                        r0 = g0 + j * P
                        x_t = bpool.tile([P, C], f32, tag="xb")
                        _dma(x_t[:], xs[r0:r0 + P, :])
                        pr_ps = pf1.tile([P, C], f32, tag="pr_ps")
                        for cc in range(CC):
                            nc.tensor.matmul(
                                pr_ps[:], atT[cc][:, j * P:(j + 1) * P],
                                wp_sb[:, cc, :], start=(cc == 0), stop=False)
                        nc.tensor.matmul(pr_ps[:], ones_r_sb[:], pb_sb[:],
                                         start=False, stop=True)
                        x1 = x1pool.tile([P, C], f32, tag="x1")
                        nc.vector.tensor_tensor(x1[:], x_t[:], pr_ps[:],
                                                op=OP.add)
                        x1_t.append(x1)
                        if debug and g == 0 and j == 0:
                            _dma(dbg["d_x1"][:], x1[:])

                        # LN2 standardization
                        st6 = spool.tile([P, 6], f32, tag="st6")
                        nc.vector.bn_stats(st6[:], x1[:])
                        mv = spool.tile([P, 2], f32, tag="mv")
                        nc.vector.bn_aggr(mv[:], st6[:])
                        rstd = spool.tile([P, 1], f32, tag="rstd")
                        nc.scalar.activation(rstd[:], mv[:, 1:2], AF.Sqrt,
                                             bias=eps_sb[:])
                        nc.vector.reciprocal(rstd[:], rstd[:])
                        h_bf = bpool.tile([P, C], bf16, tag="hb")
                        nc.vector.tensor_scalar(
                            h_bf[:], x1[:], scalar1=mv[:, 0:1], scalar2=rstd[:],
                            op0=OP.subtract, op1=OP.mult)
                        ht_ps = pht.tile([P, C], bf16)
                        for cc in range(CC):
                            nc.tensor.transpose(
                                ht_ps[:, cc * P:(cc + 1) * P],
                                h_bf[:, cc * P:(cc + 1) * P], ident_sb[:])
                        nc.scalar.copy(
                            hT_g[:, :, j * P:(j + 1) * P],
                            ht_ps[:].rearrange("p (c r) -> p c r", c=CC))

                    # mlp1: mid^T = gelu(W1^T @ h^T + c1), per hid chunk
                    midT = midpool.tile([P, HT, 512], bf16, tag="midg")
                    for t in range(HT):
                        m_ps = pmid.tile([P, 512], f32)
                        for cc in range(CC):
                            nc.tensor.matmul(
                                m_ps[:], w1_sb[:, cc, t * P:(t + 1) * P],
                                hT_g[:, cc, :], start=(cc == 0),
                                stop=(cc == CC - 1))
                        nc.scalar.activation(midT[:, t, :], m_ps[:], AF.Gelu,
                                             bias=c1_sb[:, t:t + 1])
                        if debug and g == 0 and t == 0:
                            mf = bpool.tile([P, 512], f32, tag="dmf")
                            nc.vector.tensor_copy(mf[:], midT[:, 0, :])
                            _dma(dbg["d_mid"][:], mf[:])

                    # mlp2 + residual, per subtile
                    for j in range(4):
                        r0 = g0 + j * P
                        o_ps = pf1.tile([P, C], f32, tag="o_ps")
                        for t in range(HT):
                            nc.tensor.matmul(
                                o_ps[:], midT[:, t, j * P:(j + 1) * P],
                                w2_sb[:, t, :], start=(t == 0), stop=False)
                        nc.tensor.matmul(o_ps[:], ones_r_sb[:], b2_sb[:],
                                         start=False, stop=True)
                        o_t = bpool.tile([P, C], f32, tag="ob")
                        nc.vector.tensor_tensor(o_t[:], x1_t[j][:], o_ps[:],
                                                op=OP.add)
                        _dma(out[r0:r0 + P, :], o_t[:])

    nc.finalize()
    return nc


def _host_prep(inputs):
    """Fold LN affines into weights, cast, build aux constants (numpy)."""
    from concourse import mybir
    bf = mybir.dt.np(mybir.dt.bfloat16)
    f32 = np.float32
    g = {k: np.asarray(v, f32) for k, v in inputs.items()}

    wqkv = g["qkv_w"] * g["norm1_w"][:, None]            # [C, 3C]
    c0 = g["norm1_b"] @ g["qkv_w"] + g["qkv_b"]          # [3C]
    w1 = g["mlp_w1"] * g["norm2_w"][:, None]             # [C, HID]
    c1 = g["norm2_b"] @ g["mlp_w1"] + g["mlp_b1"]        # [HID]

    feed = {
        "wq": wqkv[:, 0:C].astype(bf),
        "wkv": np.ascontiguousarray(wqkv[:, C:3 * C]).astype(bf),
        "wp": g["proj_w"].astype(bf),
        "w1": w1.astype(bf),
        "w2": g["mlp_w2"].astype(bf),
        "ckv_r": c0[C:3 * C].reshape(1, 2 * C).astype(bf),
        "cq_c": c0[0:C].reshape(H, D).T.astype(f32).copy(),
        "c1_c": c1.reshape(HT, P).T.astype(f32).copy(),
        "pb_r": g["proj_b"].reshape(1, C).astype(bf),
        "b2_r": g["mlp_b2"].reshape(1, C).astype(bf),
        "ones_r": np.ones((1, P), bf),
        "ones_c": np.ones((P, 1), bf),
        "ident": np.eye(P, dtype=f32).astype(bf),
        "swk_c": (SCALE * g["lnk_w"]).reshape(P, 1).astype(f32),
        "snbk_c": (SCALE * N * g["lnk_b"]).reshape(P, 1).astype(f32),
        "wv_c": g["lnv_w"].reshape(P, 1).astype(f32),
        "sbk_r": (SCALE * g["lnk_b"]).reshape(1, P).astype(bf),
        "bv_r": g["lnv_b"].reshape(1, P).astype(bf),
        "wv4_r": np.tile(g["lnv_w"], H).reshape(1, C).astype(bf),
    }
    return feed, g["x"]


def kernel(**inputs):
    from concourse.bass_utils import run_bass_kernel_spmd

    feed, x = _host_prep(inputs)
    if "nc" not in _cache:
        _cache["nc"] = _build_nc(NCORES)
    nc = _cache["nc"]

    in_maps = []
    for c in range(NCORES):
        b, half = c // 2, c % 2
        m = dict(feed)
        m["xs"] = np.ascontiguousarray(x[b, half * RPC:(half + 1) * RPC, :])
        in_maps.append(m)

    res = run_bass_kernel_spmd(nc, in_maps, core_ids=list(range(NCORES)))
    out = np.empty((B, N, C), np.float32)
    for c in range(NCORES):
        b, half = c // 2, c % 2
        out[b, half * RPC:(half + 1) * RPC, :] = res.results[c]["out"]
    return out
